# revision 1
# baseline (speedup 1.0000x reference)
"""MoE transformer layer (attention + top-1 routed MoE FFN) on 8 TRN2 NeuronCores.

Sharding:
  - tokens strided across cores: core c owns tokens n with n % 8 == c  (256 each)
  - attention sequence-parallel: each core computes q/k/v for its tokens,
    AllGathers k^T and v_aug (v plus an appended ones-column whose AV matmul row
    yields the softmax denominators), computes scores/AV for its query tokens
  - experts sharded 2-per-core: on-device routing (cumsum over global token
    order), dispatch via indirect-DMA scatter/gather, AllGather of expert outs
Precision:
  - attention matmul chain float32r (TF32-like), probs fp16,
    LN/softmax/router fp32, expert FFN bf16
"""
import numpy as np
import ml_dtypes

N, D, H, FF, E = 2048, 1024, 16, 4096, 16
DH = D // H           # 64
NC = 8
TPC = N // NC         # 256 tokens per core
CAP = int(1.25 * N / E)   # 160
NSLOT = 2 * CAP       # 320 slots per core
EPS = 1e-5
P = 128
DEBUG = True

_cache = {}


def _build():
    import concourse.bacc as bacc
    import concourse.bass as bass
    import concourse.mybir as mybir
    import concourse.tile as tile
    from concourse.masks import make_identity

    f32 = mybir.dt.float32
    f32r = mybir.dt.float32r
    f16 = mybir.dt.float16
    bf16 = mybir.dt.bfloat16
    i32 = mybir.dt.int32
    i16 = mybir.dt.int16
    u32 = mybir.dt.uint32
    AF = mybir.ActivationFunctionType
    OP = mybir.AluOpType
    AX = mybir.AxisListType

    nc = bacc.Bacc(None, target_bir_lowering=False, num_devices=NC)
    dp = nc.declare_dram_parameter

    # ---------------- inputs ----------------------------------------------
    x_in = dp("x_my", [TPC, D], f32, isOutput=False)            # per-core rows
    masks_in = dp("masks", [P, 16, TPC], f16, isOutput=False)   # per-core
    myn_in = dp("myn", [P, 2], i32, isOutput=False)             # per-core
    c320_in = dp("c320", [1, 1], f32, isOutput=False)           # per-core
    w1t_in = dp("w1t", [2, 8, 32, P, P], bf16, isOutput=False)  # per-core tiles
    w2t_in = dp("w2t", [2, 32, 8, P, P], bf16, isOutput=False)  # per-core tiles
    b1c_in = dp("b1c", [2, P, 32], f32, isOutput=False)         # per-core
    b2c_in = dp("b2c", [2, P, 8], f32, isOutput=False)          # per-core
    wkT_in = dp("wkT", [D, D], f32r, isOutput=False)            # shared ...
    wqT_in = dp("wqT", [D, D], f32r, isOutput=False)
    wvT_in = dp("wvT", [D, D], f32r, isOutput=False)
    ipwT_in = dp("ipwT", [D, 3 * D], f32r, isOutput=False)
    opwT_in = dp("opwT", [D, D], f32r, isOutput=False)
    rwT_in = dp("rwT", [D, E], f32, isOutput=False)
    bk_in = dp("bk", [D], f32, isOutput=False)
    bq_in = dp("bq", [D], f32, isOutput=False)
    bv_in = dp("bv", [D], f32, isOutput=False)
    ipb_in = dp("ipb", [3 * D], f32, isOutput=False)
    opb_in = dp("opb", [D], f32, isOutput=False)
    rb_in = dp("rb", [E], f32, isOutput=False)
    ln1w_in = dp("ln1w", [D], f32, isOutput=False)
    ln1b_in = dp("ln1b", [D], f32, isOutput=False)
    ln2w_in = dp("ln2w", [D], f32, isOutput=False)
    ln2b_in = dp("ln2b", [D], f32, isOutput=False)
    pic_in = dp("pic", [P, 16], i16, isOutput=False)            # shared

    out_my = dp("out_my", [TPC, D], f32, isOutput=True)
    if DEBUG:
        x2_dbg = dp("x2_dbg", [P, 2, D], f32, isOutput=True)
        h2_dbg = dp("h2_dbg", [P, 2, D], f32, isOutput=True)
        slot_dbg = dp("slot_dbg", [N], f32, isOutput=True)
        ye_dbg = dp("ye_dbg", [NSLOT, D], f32, isOutput=True)

    # ---------------- internal DRAM ---------------------------------------
    kT_agin = nc.dram_tensor("kT_agin", [D, TPC], f32)
    kT_agout = nc.dram_tensor("kT_agout", [NC * D, TPC], f32, addr_space="Shared")
    va_agin = nc.dram_tensor("va_agin", [TPC, H * 65], f16)
    va_agout = nc.dram_tensor("va_agout", [NC * TPC, H * 65], f16, addr_space="Shared")
    h2_agin = nc.dram_tensor("h2_agin", [TPC + 1, D], bf16)
    h2_agout = nc.dram_tensor("h2_agout", [NC * (TPC + 1), D], bf16, addr_space="Shared")
    ye_agin = nc.dram_tensor("ye_agin", [NSLOT, D], bf16)
    ye_agout = nc.dram_tensor("ye_agout", [NC * NSLOT, D], bf16, addr_space="Shared")
    slotrow_d = nc.dram_tensor("slotrow_d", [N], f32)
    lslrow_d = nc.dram_tensor("lslrow_d", [N], f32)
    idx_d = nc.dram_tensor("idx_d", [512], i16)

    RG = [list(range(NC))]

    from contextlib import ExitStack
    with tile.TileContext(nc, num_cores=NC) as tc, \
         tc.tile_pool(name="const", bufs=1) as cp, \
         tc.tile_pool(name="persist", bufs=1) as pp, \
         tc.tile_pool(name="small", bufs=3) as wp:

        # ---------------- constants ---------------------------------------
        ident = cp.tile([P, P], f32)
        make_identity(nc, ident)
        ident_bf = cp.tile([P, P], bf16)
        nc.vector.tensor_copy(ident_bf[:], ident[:])
        ones1x64 = cp.tile([1, 64], f32)
        nc.vector.memset(ones1x64[:], 1.0)
        ones16x1 = cp.tile([E, 1], f32)
        nc.vector.memset(ones16x1[:], 1.0)
        zeros_sb = cp.tile([P, TPC], f32)
        nc.vector.memset(zeros_sb[:], 0.0)
        eps_c = cp.tile([P, 1], f32)
        nc.vector.memset(eps_c[:], EPS)

        rb_bc = cp.tile([P, E], f32, tag="rb_bc")
        nc.sync.dma_start(rb_bc[:], rb_in[None, :].to_broadcast((P, E)))

        def bias_cols(src, width, tag):
            t = cp.tile([P, width], f32, tag=tag)
            nc.sync.dma_start(t[:], src.rearrange("(j p) -> p j", p=P))
            return t

        bk_c = bias_cols(bk_in[:], 8, "bk_c")
        bq_c = bias_cols(bq_in[:], 8, "bq_c")
        bv_c = bias_cols(bv_in[:], 8, "bv_c")
        ipb_c = bias_cols(ipb_in[:], 24, "ipb_c")
        opb_c = bias_cols(opb_in[:], 8, "opb_c")

        pic_sb = cp.tile([P, 16], i16, tag="pic_sb")
        nc.sync.dma_start(pic_sb[:], pic_in[:])
        myn_sb = cp.tile([P, 2], i32, tag="myn_sb")
        nc.sync.dma_start(myn_sb[:], myn_in[:])
        c320_sb = cp.tile([1, 1], f32, tag="c320_sb")
        nc.sync.dma_start(c320_sb[:], c320_in[:])
        rwT_sb = cp.tile([P, 8, E], f32, tag="rwT_sb")
        nc.sync.dma_start(rwT_sb[:], rwT_in[:].rearrange("(ki p) e -> p ki e", p=P))
        b1c_sb = cp.tile([P, 2, 32], f32, tag="b1c_sb")
        nc.sync.dma_start(b1c_sb[:], b1c_in[:].rearrange("e p m -> p e m"))
        b2c_sb = cp.tile([P, 2, 8], f32, tag="b2c_sb")
        nc.sync.dma_start(b2c_sb[:], b2c_in[:].rearrange("e p m -> p e m"))

        # ---------------- phase A: LN1 + transpose -------------------------
        x2_sb = pp.tile([P, 2, D], f32, tag="x2")      # starts as x, becomes x2
        qT_sb = pp.tile([P, 8, TPC], f32r, tag="qT")
        ctxT_sb = pp.tile([P, 8, TPC], f32r, tag="ctxT")
        h2_sb = pp.tile([P, 2, D], f32, tag="h2")
        h2bf_sb = pp.tile([P, 2, D], bf16, tag="h2bf")
        h2T_sb = pp.tile([P, 8, TPC], f32, tag="h2T")
        rec_sb = pp.tile([P, 2, 2], f32, tag="rec")

        def layer_norm(sp, xt, w_in, b_in, out_tile):
            w_bc = sp.tile([P, D], f32, tag="lnw")
            nc.sync.dma_start(w_bc[:], w_in[None, :].to_broadcast((P, D)))
            b_bc = sp.tile([P, D], f32, tag="lnb")
            nc.sync.dma_start(b_bc[:], b_in[None, :].to_broadcast((P, D)))
            xc = sp.tile([P, D], f32, tag="xc")
            sq_sb = sp.tile([P, D], f32, tag="sq")
            ssum = wp.tile([P, 1], f32, tag="ssum")
            nc.vector.tensor_reduce(ssum[:], xt, AX.X, OP.add)
            mu = wp.tile([P, 1], f32, tag="mu")
            nc.vector.tensor_scalar(out=mu[:], in0=ssum[:], scalar1=1.0 / D,
                                    scalar2=None, op0=OP.mult)
            nc.vector.scalar_tensor_tensor(xc[:], xt, mu[:], xt, OP.subtract, OP.bypass)
            ssq = wp.tile([P, 1], f32, tag="ssq")
            nc.scalar.activation(sq_sb[:], xc[:], AF.Square, accum_out=ssq[:])
            std = wp.tile([P, 1], f32, tag="std")
            nc.scalar.activation(std[:], ssq[:], AF.Sqrt, scale=1.0 / D, bias=eps_c[:])
            rstd = wp.tile([P, 1], f32, tag="rstd")
            nc.vector.reciprocal(rstd[:], std[:])
            nc.vector.scalar_tensor_tensor(out_tile, xc[:], rstd[:], w_bc[:],
                                           OP.mult, OP.mult)
            nc.vector.tensor_tensor(out_tile, out_tile, b_bc[:], OP.add)

        scopeAB = ExitStack()
        sab = scopeAB.enter_context(tc.tile_pool(name="scopeAB", bufs=1))
        sab2 = scopeAB.enter_context(tc.tile_pool(name="scopeAB2", bufs=2))
        sws = scopeAB.enter_context(tc.tile_pool(name="sws", bufs=3))
        psab = scopeAB.enter_context(tc.tile_pool(name="psab", bufs=2, space="PSUM"))
        hT_sb = sab.tile([P, 8, TPC], f32r, tag="hT")
        h_sb = sab.tile([P, 2, D], f32, tag="h_sb")
        for t in range(2):
            nc.sync.dma_start(x2_sb[:, t, :], x_in[t * P:(t + 1) * P, :])
            layer_norm(sab2, x2_sb[:, t, :], ln1w_in, ln1b_in, h_sb[:, t, :])
        for t in range(2):
            for j in range(8):
                tp = psab.tile([P, P], f32, tag="tp")
                nc.tensor.transpose(tp[:], h_sb[:, t, j * P:(j + 1) * P], ident[:])
                nc.vector.tensor_copy(hT_sb[:, j, t * P:(t + 1) * P], tp[:])

        # ---------------- phase B: qkv chains (feat-major, f32r) ----------
        def proj(rhs_sb, wT_dram, bias_col, bias_off, out_sb, wT_col0):
            # out[:, mo, :] = sum_ki wT[:, col0+128mo:...].T-tile @ rhs[:, ki, :] + b
            for mo in range(8):
                wtile = sws.tile([P, 8, P], f32r, tag="wtile")
                nc.sync.dma_start(
                    wtile[:],
                    wT_dram[:, wT_col0 + mo * P: wT_col0 + (mo + 1) * P]
                    .rearrange("(ki p) m -> p ki m", p=P),
                )
                acc = psab.tile([P, TPC], f32, tag="acc")
                for ki in range(8):
                    nc.tensor.matmul(acc[:], wtile[:, ki, :], rhs_sb[:, ki, :],
                                     start=(ki == 0), stop=(ki == 7))
                nc.vector.scalar_tensor_tensor(
                    out_sb[:, mo, :], acc[:], bias_col[:, bias_off + mo:bias_off + mo + 1],
                    zeros_sb[:], OP.add, OP.bypass)

        t_q = sab2.tile([P, 8, TPC], f32r, tag="inT")
        proj(hT_sb, wkT_in, bk_c, 0, t_q, 0)
        proj(t_q, ipwT_in, ipb_c, 0, qT_sb, 0)
        t_k = sab2.tile([P, 8, TPC], f32r, tag="inT")
        proj(hT_sb, wqT_in, bq_c, 0, t_k, 0)
        kT_sb = sab2.tile([P, 8, TPC], f32, tag="kvT")
        proj(t_k, ipwT_in, ipb_c, 8, kT_sb, D)
        t_v = sab2.tile([P, 8, TPC], f32r, tag="inT")
        proj(hT_sb, wvT_in, bv_c, 0, t_v, 0)
        vT_sb = sab2.tile([P, 8, TPC], f32, tag="kvT")
        proj(t_v, ipwT_in, ipb_c, 16, vT_sb, 2 * D)

        nc.sync.dma_start(kT_agin[:].rearrange("(j p) n -> p j n", p=P), kT_sb[:])
        va_sb = sab.tile([P, 2, H, 65], f16, tag="va")
        nc.vector.memset(va_sb[:], 0.0)
        nc.vector.memset(va_sb[:, :, :, 64], 1.0)
        for t in range(2):
            for j in range(8):
                tp = psab.tile([P, P], f32, tag="tp")
                nc.tensor.transpose(tp[:], vT_sb[:, j, t * P:(t + 1) * P], ident[:])
                nc.vector.tensor_copy(
                    va_sb[:, t, 2 * j:2 * j + 2, 0:64],
                    tp[:].rearrange("p (a b) -> p a b", a=2),
                )
        nc.sync.dma_start(
            va_agin[:].rearrange("(t p) w -> p t w", p=P),
            va_sb[:].rearrange("p t h w -> p t (h w)"),
        )

        # ---------------- AG1 ---------------------------------------------
        nc.gpsimd.collective_compute("AllGather", OP.bypass, replica_groups=RG,
                                     ins=[kT_agin[:]], outs=[kT_agout[:]])
        nc.gpsimd.collective_compute("AllGather", OP.bypass, replica_groups=RG,
                                     ins=[va_agin[:]], outs=[va_agout[:]])
        scopeAB.close()

        # ---------------- phase C: scores / AV per feat-tile j ------------
        scopeC = ExitStack()
        scd = scopeC.enter_context(tc.tile_pool(name="scopeC", bufs=1))
        kvp = scopeC.enter_context(tc.tile_pool(name="kvp", bufs=2))
        cws = scopeC.enter_context(tc.tile_pool(name="cws", bufs=3))
        psc = scopeC.enter_context(tc.tile_pool(name="psc", bufs=2, space="PSUM"))
        masks_sb = scd.tile([P, 16, TPC], f16, tag="masks_sb")
        nc.sync.dma_start(masks_sb[:], masks_in[:])
        for j in range(8):
            kTj = kvp.tile([P, 8, TPC], f32r, tag="kTj")
            nc.sync.dma_start(
                kTj[:],
                kT_agout[:]
                .rearrange("(r j p) n -> p j r n", p=P, j=8)[:, j, :, :]
                .bitcast(f32r),
            )
            vaj = kvp.tile([P, 16, 130], f16, tag="vaj")
            nc.sync.dma_start(
                vaj[:],
                va_agout[:].rearrange("(r kh p) w -> p (r kh) w", p=P, kh=2)
                [:, :, 65 * 2 * j: 65 * 2 * j + 130],
            )
            for hh in range(2):
                pl, pu = 64 * hh, 64 * hh + 64
                caug = psc.tile([65, TPC], f32, tag="caug")
                for rk in range(16):
                    r, kh = rk % 8, rk // 8
                    sc = psc.tile([P, TPC], f32, tag="sc")
                    nc.tensor.matmul(
                        sc[:], kTj[pl:pu, r, kh * P:(kh + 1) * P],
                        qT_sb[pl:pu, j, :], start=True, stop=True)
                    ex = cws.tile([P, TPC], f16, tag="ex")
                    nc.scalar.activation(ex[:], sc[:], AF.Exp, scale=0.125)
                    nc.vector.tensor_tensor(ex[:], ex[:],
                                            masks_sb[:, kh * 8 + r, :], OP.mult)
                    nc.tensor.matmul(
                        caug[:], vaj[:, 2 * r + kh, 65 * hh:65 * hh + 65], ex[:],
                        start=(rk == 0), stop=(rk == 15))
                rc = cws.tile([1, TPC], f32, tag="rc")
                nc.vector.reciprocal(rc[:], caug[64:65, :])
                bc = psc.tile([64, TPC], f32, tag="bc")
                nc.tensor.matmul(bc[:], ones1x64[:], rc[:], start=True, stop=True)
                bcs = cws.tile([64, TPC], f32, tag="bcs")
                nc.vector.tensor_copy(bcs[:], bc[:])
                nc.vector.tensor_tensor(ctxT_sb[pl:pu, j, :], caug[0:64, :], bcs[:],
                                        OP.mult)

        # ---------------- phase D: out-proj + residual + LN2 + router ------
        scopeC.close()
        scopeD = ExitStack()
        dws = scopeD.enter_context(tc.tile_pool(name="dws", bufs=3))
        psd = scopeD.enter_context(tc.tile_pool(name="psd", bufs=2, space="PSUM"))
        for mo in range(8):
            wtile = dws.tile([P, 8, P], f32r, tag="wtile")
            nc.sync.dma_start(
                wtile[:],
                opwT_in[:, mo * P:(mo + 1) * P].rearrange("(ki p) m -> p ki m", p=P),
            )
            acc = psd.tile([P, TPC], f32, tag="acc")
            for ki in range(8):
                nc.tensor.matmul(acc[:], wtile[:, ki, :], ctxT_sb[:, ki, :],
                                 start=(ki == 0), stop=(ki == 7))
            ao = dws.tile([P, TPC], f32, tag="ao")
            nc.scalar.activation(ao[:], acc[:], AF.Identity, bias=opb_c[:, mo:mo + 1])
            for t in range(2):
                tp = psd.tile([P, P], f32, tag="tp")
                nc.tensor.transpose(tp[:], ao[:, t * P:(t + 1) * P], ident[:])
                nc.vector.tensor_tensor(
                    x2_sb[:, t, mo * P:(mo + 1) * P],
                    x2_sb[:, t, mo * P:(mo + 1) * P], tp[:], OP.add)
        if DEBUG:
            nc.sync.dma_start(x2_dbg[:], x2_sb[:])

        for t in range(2):
            layer_norm(dws, x2_sb[:, t, :], ln2w_in, ln2b_in, h2_sb[:, t, :])
            nc.vector.tensor_copy(h2bf_sb[:, t, :], h2_sb[:, t, :])
            nc.sync.dma_start(h2_agin[t * P:(t + 1) * P, :], h2bf_sb[:, t, :])
            for j in range(8):
                tp = psd.tile([P, P], f32, tag="tp")
                nc.tensor.transpose(tp[:], h2_sb[:, t, j * P:(j + 1) * P], ident[:])
                nc.vector.tensor_copy(h2T_sb[:, j, t * P:(t + 1) * P], tp[:])
        if DEBUG:
            nc.sync.dma_start(h2_dbg[:], h2_sb[:])

        for t in range(2):
            lg = psd.tile([P, E], f32, tag="lg")
            for ki in range(8):
                nc.tensor.matmul(lg[:], h2T_sb[:, ki, t * P:(t + 1) * P],
                                 rwT_sb[:, ki, :], start=(ki == 0), stop=(ki == 7))
            lgs = wp.tile([P, E], f32, tag="lgs")
            nc.vector.tensor_tensor(lgs[:], lg[:], rb_bc[:], OP.add)
            nlmax = wp.tile([P, 1], f32, tag="nlmax")
            nc.vector.tensor_reduce(nlmax[:], lgs[:], AX.X, OP.max, negate=True)
            exl = wp.tile([P, E], f32, tag="exl")
            sumexp = wp.tile([P, 1], f32, tag="sumexp")
            nc.scalar.activation(exl[:], lgs[:], AF.Exp, bias=nlmax[:],
                                 accum_out=sumexp[:])
            nc.vector.reciprocal(rec_sb[:, t, 1:2], sumexp[:])
            mx8 = wp.tile([P, 8], f32, tag="mx8")
            mi8 = wp.tile([P, 8], u32, tag="mi8")
            nc.vector.max(mx8[:], lgs[:])
            nc.vector.max_index(mi8[:], mx8[:], lgs[:])
            nc.vector.tensor_copy(rec_sb[:, t, 0:1], mi8[:, 0:1])
            nc.sync.dma_start(
                h2_agin[TPC, :].bitcast(f32)
                .rearrange("(t p c) -> t p c", t=2, c=2)[t],
                rec_sb[:, t, :],
            )

        # ---------------- AG2 ----------------------------------------------
        nc.gpsimd.collective_compute("AllGather", OP.bypass, replica_groups=RG,
                                     ins=[h2_agin[:]], outs=[h2_agout[:]])
        scopeD.close()

        # ---------------- phase E: routing rows (replicated) ----------------
        scopeEF = ExitStack()
        sef = scopeEF.enter_context(tc.tile_pool(name="scopeEF", bufs=1))
        erow = scopeEF.enter_context(tc.tile_pool(name="erow", bufs=3))
        ews = scopeEF.enter_context(tc.tile_pool(name="ews", bufs=3))
        ew2 = scopeEF.enter_context(tc.tile_pool(name="ew2", bufs=2))
        pse = scopeEF.enter_context(tc.tile_pool(name="pse", bufs=2, space="PSUM"))
        recview = (h2_agout[:].bitcast(f32)
                   .rearrange("(r a) w -> r a w", a=TPC + 1)[:, TPC, :])  # [8, 512]
        routes_bc = sef.tile([E, N], f32, tag="routes_bc")
        for r in range(8):
            nc.sync.dma_start(
                routes_bc[:].rearrange("p (i r) -> p i r", r=8)[:, :, r],
                recview[r:r + 1, 0::2].to_broadcast((E, TPC)),
            )
        ecol = cp.tile([E, 1], i32, tag="ecol")
        nc.gpsimd.iota(ecol[:], pattern=[[0, 1]], base=0, channel_multiplier=1)
        ecolf = cp.tile([E, 1], f32, tag="ecolf")
        nc.vector.tensor_copy(ecolf[:], ecol[:])
        oh = erow.tile([E, N], f32, tag="ohrow")
        nc.vector.tensor_tensor(oh[:], routes_bc[:], ecolf[:].to_broadcast((E, N)),
                                OP.is_equal)
        cs = erow.tile([E, N], f32, tag="ohrow")
        nc.vector.tensor_tensor_scan(cs[:], oh[:], oh[:], 0.0, OP.add, OP.bypass)
        pm1 = erow.tile([E, N], f32, tag="ohrow")
        nc.vector.scalar_tensor_tensor(pm1[:], cs[:], 1.0, oh[:], OP.subtract,
                                       OP.mult)
        posr = erow.tile([1, N], f32, tag="rowf")
        for q in range(4):
            pq = pse.tile([1, 512], f32, tag="pq")
            nc.tensor.matmul(pq[:], ones16x1[:], pm1[:, q * 512:(q + 1) * 512],
                             start=True, stop=True)
            nc.vector.tensor_copy(posr[:, q * 512:(q + 1) * 512], pq[:])
        slotr = erow.tile([1, N], f32, tag="rowf")
        nc.vector.scalar_tensor_tensor(slotr[:], routes_bc[0:1, :], float(CAP),
                                       posr[:], OP.mult, OP.add)
        keepr = erow.tile([1, N], f32, tag="rowf")
        nc.vector.tensor_scalar(out=keepr[:], in0=posr[:], scalar1=float(CAP),
                                scalar2=None, op0=OP.is_lt)
        nc.vector.scalar_tensor_tensor(slotr[:], slotr[:], float(E * CAP), keepr[:],
                                       OP.subtract, OP.mult)
        nc.vector.tensor_scalar(out=slotr[:], in0=slotr[:], scalar1=float(E * CAP),
                                scalar2=None, op0=OP.add)
        nc.sync.dma_start(slotrow_d[:].unsqueeze(0), slotr[:])
        if DEBUG:
            nc.sync.dma_start(slot_dbg[:].unsqueeze(0), slotr[:])
        lslr = erow.tile([1, N], f32, tag="rowf")
        nc.vector.scalar_tensor_tensor(lslr[:], slotr[:], c320_sb[:, 0:1], slotr[:],
                                       OP.subtract, OP.bypass)
        nc.sync.dma_start(lslrow_d[:].unsqueeze(0), lslr[:])

        zt = wp.tile([P, 4], i16, tag="zt")
        nc.vector.memset(zt[:], 0)
        nc.sync.dma_start(idx_d[:].rearrange("(g p) -> p g", p=P), zt[:])
        lcolf = wp.tile([P, 16], f32, tag="lcolf")
        nc.sync.dma_start(lcolf[:], lslrow_d[:].rearrange("(k p) -> p k", p=P))
        lcol = wp.tile([P, 16], i32, tag="lcol")
        nc.vector.tensor_copy(lcol[:], lcolf[:])
        for k in range(16):
            nc.gpsimd.indirect_dma_start(
                out=idx_d[:, None],
                out_offset=bass.IndirectOffsetOnAxis(ap=lcol[:, k:k + 1], axis=0),
                in_=pic_sb[:, k:k + 1],
                in_offset=None,
                bounds_check=NSLOT - 1,
                oob_is_err=False,
            )
        idxc16 = wp.tile([P, 4], i16, tag="idxc16")
        nc.sync.dma_start(idxc16[:], idx_d[:].rearrange("(g p) -> p g", p=P))
        idxc = wp.tile([P, 4], i32, tag="idxc")
        nc.vector.tensor_copy(idxc[:], idxc16[:])

        xeT_sb = sef.tile([P, 8, 3 * P], bf16, tag="xeT")
        for g in range(3):
            xg = ew2.tile([P, D], bf16, tag="xg")
            nc.gpsimd.indirect_dma_start(
                out=xg[:],
                out_offset=None,
                in_=h2_agout[:],
                in_offset=bass.IndirectOffsetOnAxis(ap=idxc[:, g:g + 1], axis=0),
                bounds_check=NC * (TPC + 1) - 1,
                oob_is_err=False,
            )
            for j in range(8):
                tp = pse.tile([P, P], bf16, tag="tpb")
                nc.tensor.transpose(tp[:], xg[:, j * P:(j + 1) * P], ident_bf[:])
                nc.vector.tensor_copy(xeT_sb[:, j, g * P:(g + 1) * P], tp[:])

        # ---------------- phase F: experts (bf16) ---------------------------
        yeT_sb = sef.tile([P, 8, NSLOT], bf16, tag="yeT")
        h1T_sb = sef.tile([P, 32, CAP], bf16, tag="h1T")
        for el in range(2):
            s0 = el * CAP
            for mo in range(32):
                w1tile = ews.tile([P, 8, P], bf16, tag="w1tile")
                nc.sync.dma_start(w1tile[:],
                                  w1t_in[el, :, mo, :, :].rearrange("k p m -> p k m"))
                acc = pse.tile([P, CAP], f32, tag="eacc")
                for ki in range(8):
                    nc.tensor.matmul(acc[:], w1tile[:, ki, :],
                                     xeT_sb[:, ki, s0:s0 + CAP],
                                     start=(ki == 0), stop=(ki == 7))
                nc.vector.scalar_tensor_tensor(
                    h1T_sb[:, mo, :], acc[:], b1c_sb[:, el, mo:mo + 1],
                    zeros_sb[:, :CAP], OP.add, OP.max)
            for mo in range(8):
                acc = pse.tile([P, CAP], f32, tag="eacc")
                for kb in range(4):
                    w2tile = ews.tile([P, 8, P], bf16, tag="w2tile")
                    nc.sync.dma_start(
                        w2tile[:],
                        w2t_in[el, kb * 8:(kb + 1) * 8, mo, :, :]
                        .rearrange("k p m -> p k m"))
                    for kk in range(8):
                        ki = kb * 8 + kk
                        nc.tensor.matmul(acc[:], w2tile[:, kk, :], h1T_sb[:, ki, :],
                                         start=(ki == 0), stop=(ki == 31))
                nc.vector.scalar_tensor_tensor(
                    yeT_sb[:, mo, s0:s0 + CAP], acc[:], b2c_sb[:, el, mo:mo + 1],
                    zeros_sb[:, :CAP], OP.add, OP.bypass)

        for g in range(3):
            cols = P if g < 2 else NSLOT - 2 * P  # 128,128,64
            yeg = ew2.tile([P, D], bf16, tag="yeg")
            for j in range(8):
                tp = pse.tile([P, P], bf16, tag="tpb")
                nc.tensor.transpose(tp[:cols, :], yeT_sb[:, j, g * P:g * P + cols],
                                    ident_bf[:])
                nc.vector.tensor_copy(yeg[:cols, j * P:(j + 1) * P], tp[:cols, :])
            nc.sync.dma_start(ye_agin[g * P:g * P + cols, :], yeg[:cols, :])
            if DEBUG:
                yegf = ew2.tile([P, D], f32, tag="yegf")
                nc.vector.tensor_copy(yegf[:cols, :], yeg[:cols, :])
                nc.sync.dma_start(ye_dbg[g * P:g * P + cols, :], yegf[:cols, :])

        # ---------------- AG3 ----------------------------------------------
        nc.gpsimd.collective_compute("AllGather", OP.bypass, replica_groups=RG,
                                     ins=[ye_agin[:]], outs=[ye_agout[:]])

        # ---------------- phase G: combine ----------------------------------
        for t in range(2):
            msl_f = wp.tile([P, 1], f32, tag="msl_f")
            nc.gpsimd.indirect_dma_start(
                out=msl_f[:],
                out_offset=None,
                in_=slotrow_d[:, None],
                in_offset=bass.IndirectOffsetOnAxis(ap=myn_sb[:, t:t + 1], axis=0),
                bounds_check=N - 1,
                oob_is_err=False,
            )
            msl = wp.tile([P, 1], i32, tag="msl")
            nc.vector.tensor_copy(msl[:], msl_f[:])
            yt = ew2.tile([P, D], bf16, tag="yt")
            nc.vector.tensor_copy(yt[:], h2bf_sb[:, t, :])
            nc.gpsimd.indirect_dma_start(
                out=yt[:],
                out_offset=None,
                in_=ye_agout[:],
                in_offset=bass.IndirectOffsetOnAxis(ap=msl[:, 0:1], axis=0),
                bounds_check=E * CAP - 1,
                oob_is_err=False,
            )
            ot = ew2.tile([P, D], f32, tag="ot")
            nc.vector.scalar_tensor_tensor(ot[:], yt[:], rec_sb[:, t, 1:2],
                                           x2_sb[:, t, :], OP.mult, OP.add)
            nc.sync.dma_start(out_my[t * P:(t + 1) * P, :], ot[:])
        scopeEF.close()

    nc.compile()
    return nc


def _host_prep(inputs):
    bf16 = ml_dtypes.bfloat16
    x = np.ascontiguousarray(np.asarray(inputs["x"]), dtype=np.float32)
    shared = {
        "wkT": np.ascontiguousarray(np.asarray(inputs["wk"]).T.astype(np.float32)),
        "wqT": np.ascontiguousarray(np.asarray(inputs["wq"]).T.astype(np.float32)),
        "wvT": np.ascontiguousarray(np.asarray(inputs["wv"]).T.astype(np.float32)),
        "ipwT": np.ascontiguousarray(np.asarray(inputs["ipw"]).T.astype(np.float32)),
        "opwT": np.ascontiguousarray(np.asarray(inputs["opw"]).T.astype(np.float32)),
        "rwT": np.ascontiguousarray(np.asarray(inputs["router_w"]).T.astype(np.float32)),
        "bk": np.asarray(inputs["bk"], dtype=np.float32),
        "bq": np.asarray(inputs["bq"], dtype=np.float32),
        "bv": np.asarray(inputs["bv"], dtype=np.float32),
        "ipb": np.asarray(inputs["ipb"], dtype=np.float32),
        "opb": np.asarray(inputs["opb"], dtype=np.float32),
        "rb": np.asarray(inputs["router_b"], dtype=np.float32),
        "ln1w": np.asarray(inputs["ln1_w"], dtype=np.float32),
        "ln1b": np.asarray(inputs["ln1_b"], dtype=np.float32),
        "ln2w": np.asarray(inputs["ln2_w"], dtype=np.float32),
        "ln2b": np.asarray(inputs["ln2_b"], dtype=np.float32),
    }
    pvec = np.arange(P)
    pic = ((257 * (pvec % 8))[:, None] + 16 * np.arange(16)[None, :]
           + (pvec // 8)[:, None])
    shared["pic"] = pic.astype(np.int16)

    w1 = np.asarray(inputs["w1"])
    w2 = np.asarray(inputs["w2"])
    b1 = np.asarray(inputs["b1"], dtype=np.float32)
    b2 = np.asarray(inputs["b2"], dtype=np.float32)

    in_maps = []
    for c in range(NC):
        m = dict(shared)
        m["x_my"] = np.ascontiguousarray(x[c::NC])
        i = np.arange(TPC)[None, None, :]
        rk = np.arange(16)[None, :, None]
        r_, kh_ = rk % 8, rk // 8
        p_ = pvec[:, None, None]
        kg = r_ + 8 * (P * kh_ + p_)
        qg = c + 8 * i
        m["masks"] = (kg <= qg).astype(np.float16)
        m["myn"] = (c + 8 * (P * np.arange(2)[None, :] + pvec[:, None])).astype(np.int32)
        m["c320"] = np.full((1, 1), 320.0 * c, np.float32)
        w1t = np.empty((2, 8, 32, P, P), bf16)
        w2t = np.empty((2, 32, 8, P, P), bf16)
        b1c = np.empty((2, P, 32), np.float32)
        b2c = np.empty((2, P, 8), np.float32)
        for el in range(2):
            e = 2 * c + el
            w1T = np.ascontiguousarray(w1[e].T).astype(bf16)   # [D, F]
            w2T = np.ascontiguousarray(w2[e].T).astype(bf16)   # [F, D]
            w1t[el] = w1T.reshape(8, P, 32, P).transpose(0, 2, 1, 3)
            w2t[el] = w2T.reshape(32, P, 8, P).transpose(0, 2, 1, 3)
            b1c[el] = b1[e].reshape(32, P).T
            b2c[el] = b2[e].reshape(8, P).T
        m["w1t"], m["w2t"], m["b1c"], m["b2c"] = w1t, w2t, b1c, b2c
        in_maps.append(m)
    return in_maps


def kernel(**inputs):
    from concourse.bass_utils import run_bass_kernel_spmd

    if "nc" not in _cache:
        _cache["nc"] = _build()
    nc = _cache["nc"]
    in_maps = _host_prep(inputs)
    res = run_bass_kernel_spmd(nc, in_maps, list(range(NC)))
    out = np.zeros((N, D), np.float32)
    for c in range(NC):
        out[c::NC] = res.results[c]["out_my"]
    _cache["results"] = res.results
    return out

